# revision 1
# baseline (speedup 1.0000x reference)
"""GIN message-passing network on 8 Trainium2 NeuronCores.

Strategy
--------
The edge list is converted (on host, O(E)) to a dense transposed adjacency
matrix A_T[src, dst].  segment_sum(h[src], dst) == A @ h, and since the GIN
update is ((1+eps)h + A@h) @ W = (1+eps)(h@W) + A@(h@W), the N x N adjacency
contraction runs at hidden width H=512 instead of 4096.  The (1+eps) self term
is folded into A's diagonal on host (per layer, eps is an input).  BatchNorm
(eval mode) + bias are folded on host into per-feature affine scale/shift
applied with fused scalar-engine relu-activations.

Sharding: nodes are split 8 ways (512 nodes per core).  Activations are kept
feature-major ("transposed", [H, nodes]) on each core so that per-feature
affines land on the partition axis and no on-device transposes are needed:

  Z_c   = matmul(lhsT=hT_c,  rhs=Wa)     -> [nodes_c, H] (node-major)
  AllGather(Z_c)                         -> Zfull [N, H]
  XT_c  = matmul(lhsT=Zfull, rhs=AT_c)   -> [H, nodes_c] (incl. (1+eps)Z diag)
  YT_c  = matmul(lhsT=Wb,    rhs=XT_c)   -> [H, nodes_c] = next hT_c

The jumping-knowledge readout (huge [N*N,2]/[N*H,2] linears) is an
elementwise-product reduction done on the vector engine with
tensor_tensor_reduce against host-relaid-out weight shards, finished with a
[128,2]x[128,1] partition-reduce matmul and a tiny AllReduce.

All matmul operands are bf16 (fp32 PSUM accumulation); measured end-to-end
relative error vs the fp32 reference is ~0.6%.
"""

import numpy as np
import ml_dtypes

import concourse.bass as bass
import concourse.bacc as bacc
import concourse.tile as tile
import concourse.mybir as mybir
from concourse.bass_utils import run_bass_kernel_spmd

bf16 = ml_dtypes.bfloat16
dt = mybir.dt
AF = mybir.ActivationFunctionType
ALU = mybir.AluOpType

N_FULL, H_FULL, C, NL, NCORES = 4096, 512, 2, 5, 8
NLAY = NL - 1  # 4 GIN layers


def build_program(N=N_FULL, H=H_FULL, ncores=NCORES, reps=1, use_coll=True, use_readout=True, at_fp8=False, use_ar=True):
    """Emit the SPMD Bass program (same program on all cores).

    reps > 1 repeats the whole computation (for slope-based timing)."""
    NPC = N // ncores          # nodes per core
    KT0 = N // 128             # k-tiles for layer-0 MLP / adjacency contraction
    HT = H // 128              # tiles over hidden dim
    MT = NPC // 128            # tiles over this core's nodes
    NSLOT = KT0 + NLAY * HT    # readout accumulator slots per class

    nc = bacc.Bacc("TRN2", target_bir_lowering=False, debug=False,
                   num_devices=ncores)

    featT = nc.dram_tensor("featT", [N, NPC], dt.bfloat16, kind="ExternalInput")
    w0a = nc.dram_tensor("w0a", [N, H], dt.bfloat16, kind="ExternalInput")
    wra = nc.dram_tensor("wra", [NLAY - 1, H, H], dt.bfloat16, kind="ExternalInput")
    wb = nc.dram_tensor("wb", [NLAY, H, H], dt.bfloat16, kind="ExternalInput")
    at_dt = dt.float8e4 if at_fp8 else dt.bfloat16
    at = nc.dram_tensor("at", [NLAY, N, NPC], at_dt, kind="ExternalInput")
    wp0 = nc.dram_tensor("wp0", [C, N, NPC], dt.bfloat16, kind="ExternalInput")
    wpr = nc.dram_tensor("wpr", [NLAY, C, H, NPC], dt.bfloat16, kind="ExternalInput")
    aff = nc.dram_tensor("aff", [128, NLAY * 6 * HT], dt.float32, kind="ExternalInput")
    zdummy = (None if use_coll else
              nc.dram_tensor("zdummy", [N, H], dt.bfloat16, kind="ExternalInput"))
    bias = nc.dram_tensor("bias", [C, 1], dt.float32, kind="ExternalInput")
    score = nc.dram_tensor("score", [C, 1], dt.float32, kind="ExternalOutput")

    rg = [list(range(ncores))]

    def aff_col(lay, stage, m):
        return lay * 6 * HT + stage * HT + m

    with tile.TileContext(nc) as tc:
        with (
            tc.tile_pool(name="dram", bufs=2, space="DRAM") as dram,
            tc.tile_pool(name="big", bufs=1) as big,
            tc.tile_pool(name="sb", bufs=2) as sb,
            tc.tile_pool(name="stream", bufs=4) as stream,
            tc.tile_pool(name="acc", bufs=8, space="PSUM") as psum,
        ):
            # -- constants / small resident tensors
            aff_sb = big.tile([128, NLAY * 6 * HT], dt.float32, tag="aff")
            nc.sync.dma_start(aff_sb[:], aff[:])
            racc = big.tile([128, C * NSLOT], dt.float32, tag="racc")
            if not use_readout:
                nc.vector.memset(racc[:], 0.0)

            # -- feat (transposed, this core's node columns) resident in SBUF.
            # Chunks are DMA'd inside layer 0's k-loop so the first matmul
            # starts as soon as chunk 0 lands.
            CH = 8  # k-tiles per DMA chunk
            featT_sb = big.tile([128, KT0, NPC], dt.bfloat16, tag="featT")

            def load_featT_chunk(k0):
                cc = min(CH, KT0 - k0)
                nc.sync.dma_start(
                    featT_sb[:, k0:k0 + cc, :],
                    featT[k0 * 128:(k0 + cc) * 128, :].rearrange("(t p) h -> p t h", p=128))

            for rep in range(reps):
              # -- feat readout: score_c += sum(featT * Wp0rT[c]).
              # Emitted after layer 0 so its DMA stream doesn't starve the
              # layer-0 matmul weight loads (scheduler priority ~ emission
              # order).
              def emit_feat_readout(half):
                  for c in range(C if use_readout else 0):
                      lo = 0 if half == 0 else KT0 // 2
                      hi = KT0 // 2 if half == 0 else KT0
                      for k0 in range(lo, hi, CH):
                          cc = min(CH, KT0 - k0)
                          wt = stream.tile([128, CH, NPC], dt.bfloat16, tag="wro",
                                           bufs=2, name="wt")
                          nc.sync.dma_start(
                              wt[:, :cc, :],
                              wp0[c, k0 * 128:(k0 + cc) * 128, :].rearrange("(t p) h -> p t h", p=128))
                          for kk in range(cc):
                              k = k0 + kk
                              scr = stream.tile([128, NPC], dt.float32, tag="scr",
                                                name="scr")
                              nc.vector.scalar_tensor_tensor(
                                  out=scr[:], in0=featT_sb[:, k, :], scalar=1.0,
                                  in1=wt[:, kk, :], op0=ALU.mult, op1=ALU.mult,
                                  accum_out=racc[:, c * NSLOT + k: c * NSLOT + k + 1])

              hT_sb = None
              for lay in range(NLAY):
                  K = N if lay == 0 else H
                  KT = K // 128

                  # ---- Z_c = h_c @ Wa  (node-major out [NPC, H]) ----
                  psZ = [psum.tile([128, H], dt.float32, tag="acc", name=f"psZ{m}") for m in range(MT)]
                  wsrc = w0a if lay == 0 else wra[lay - 1]
                  for k0 in range(0, KT, CH):
                      cc = min(CH, KT - k0)
                      if lay == 0 and rep == 0:
                          load_featT_chunk(k0)
                      wa_t = stream.tile([128, CH, H], dt.bfloat16, tag="wa", bufs=2)
                      nc.sync.dma_start(
                          wa_t[:, :cc, :],
                          wsrc[k0 * 128:(k0 + cc) * 128, :].rearrange("(t p) h -> p t h", p=128))
                      for kk in range(cc):
                          k = k0 + kk
                          lhs_tile = featT_sb[:, k, :] if lay == 0 else hT_sb[:, k, :]
                          for m in range(MT):
                              nc.tensor.matmul(
                                  psZ[m][:], lhsT=lhs_tile[:, m * 128:(m + 1) * 128],
                                  rhs=wa_t[:, kk, :], start=(k == 0), stop=(k == KT - 1))

                  zin = dram.tile([NPC, H], dt.bfloat16, tag="zin")
                  zcat = sb.tile([128, MT, H], dt.bfloat16, tag="zcat")
                  for m in range(MT):
                      nc.vector.tensor_copy(zcat[:, m, :], psZ[m][:])
                  nc.sync.dma_start(
                      zin.rearrange("(m p) h -> p m h", p=128), zcat[:])

                  # ---- AllGather Z across cores -> Zfull [N, H] ----
                  zf_sb = sb.tile([128, KT0, H], dt.bfloat16, tag="zf", bufs=1)
                  zsrc = zdummy
                  if use_coll:
                      zfull = dram.tile([N, H], dt.bfloat16, tag="zfull", addr_space="Shared")
                      nc.gpsimd.collective_compute(
                          "AllGather", ALU.bypass, replica_groups=rg,
                          ins=[zin.opt()], outs=[zfull.opt()])
                      zsrc = zfull
                  for t0 in range(0, KT0, CH):
                      cc = min(CH, KT0 - t0)
                      nc.sync.dma_start(
                          zf_sb[:, t0:t0 + cc, :],
                          zsrc[t0 * 128:(t0 + cc) * 128, :].rearrange("(t p) h -> p t h", p=128))

                  # ---- XT_c = Zfull.T @ (A_T + (1+eps)I)_c  -> [H, NPC] ----
                  psX = [psum.tile([128, NPC], dt.float32, tag="acc", name=f"psX{m}") for m in range(HT)]
                  for k0 in range(0, KT0, CH):
                      cc = min(CH, KT0 - k0)
                      at_t = stream.tile([128, CH, NPC], at_dt, tag="at", bufs=2)
                      nc.sync.dma_start(
                          at_t[:, :cc, :],
                          at[lay, k0 * 128:(k0 + cc) * 128, :].rearrange("(t p) h -> p t h", p=128))
                      for kk in range(cc):
                          k = k0 + kk
                          for m in range(HT):
                              nc.tensor.matmul(
                                  psX[m][:], lhsT=zf_sb[:, k, m * 128:(m + 1) * 128],
                                  rhs=at_t[:, kk, :], start=(k == 0), stop=(k == KT0 - 1))

                  # ---- stage 1: relu(X * s1 + t1), cast bf16 ----
                  xt_sb = sb.tile([128, HT, NPC], dt.bfloat16, tag="xt")
                  for m in range(HT):
                      nc.scalar.activation(
                          xt_sb[:, m, :], psX[m][:], AF.Relu,
                          bias=aff_sb[:, aff_col(lay, 1, m):aff_col(lay, 1, m) + 1],
                          scale=aff_sb[:, aff_col(lay, 0, m):aff_col(lay, 0, m) + 1])

                  # ---- YT_c = Wb.T @ XT_c -> [H, NPC] ----
                  wb_sb = sb.tile([128, HT, H], dt.bfloat16, tag="wb")
                  nc.sync.dma_start(
                      wb_sb[:], wb[lay].rearrange("(t p) h -> p t h", p=128))
                  psY = [psum.tile([128, NPC], dt.float32, tag="acc", name=f"psY{m}") for m in range(HT)]
                  for k in range(HT):
                      for m in range(HT):
                          nc.tensor.matmul(
                              psY[m][:], lhsT=wb_sb[:, k, m * 128:(m + 1) * 128],
                              rhs=xt_sb[:, k, :], start=(k == 0), stop=(k == HT - 1))

                  # ---- stages 2+3: two fused affine+relu, cast bf16 ----
                  hT_sb = sb.tile([128, HT, NPC], dt.bfloat16, tag="hT")
                  for m in range(HT):
                      tmp = sb.tile([128, NPC], dt.float32, tag="tmp")
                      nc.scalar.activation(
                          tmp[:], psY[m][:], AF.Relu,
                          bias=aff_sb[:, aff_col(lay, 3, m):aff_col(lay, 3, m) + 1],
                          scale=aff_sb[:, aff_col(lay, 2, m):aff_col(lay, 2, m) + 1])
                      nc.scalar.activation(
                          hT_sb[:, m, :], tmp[:], AF.Relu,
                          bias=aff_sb[:, aff_col(lay, 5, m):aff_col(lay, 5, m) + 1],
                          scale=aff_sb[:, aff_col(lay, 4, m):aff_col(lay, 4, m) + 1])

                  if lay in (0, 1):
                      emit_feat_readout(lay)

                  # ---- readout for this hidden rep ----
                  for c in range(C if use_readout else 0):
                      wt = stream.tile([128, HT, NPC], dt.bfloat16, tag="wrr", bufs=2)
                      nc.sync.dma_start(
                          wt[:], wpr[lay, c].rearrange("(t p) h -> p t h", p=128))
                      for m in range(HT):
                          scr = stream.tile([128, NPC], dt.float32, tag="scr")
                          slot = c * NSLOT + KT0 + lay * HT + m
                          nc.vector.scalar_tensor_tensor(
                              out=scr[:], in0=hT_sb[:, m, :], scalar=1.0,
                              in1=wt[:, m, :], op0=ALU.mult, op1=ALU.mult,
                              accum_out=racc[:, slot:slot + 1])

              # ---- finish readout: partition reduce + AllReduce + bias ----
              r2 = sb.tile([128, C], dt.float32, tag="r2")
              for c in range(C):
                  nc.vector.tensor_reduce(
                      r2[:, c:c + 1], racc[:, c * NSLOT:(c + 1) * NSLOT],
                      axis=mybir.AxisListType.X, op=ALU.add)
              ones = sb.tile([128, 1], dt.float32, tag="ones")
              nc.vector.memset(ones[:], 1.0)
              psS = psum.tile([C, 1], dt.float32, tag="acc")
              nc.tensor.matmul(psS[:], lhsT=r2[:], rhs=ones[:], start=True, stop=True)
              s_sb = sb.tile([C, 1], dt.float32, tag="s_sb")
              nc.vector.tensor_copy(s_sb[:], psS[:])
              part = dram.tile([C, 1], dt.float32, tag="part")
              nc.sync.dma_start(part[:], s_sb[:])
              ar_sb = sb.tile([C, 1], dt.float32, tag="ar_sb")
              if use_coll and use_ar:
                  ar_out = dram.tile([C, 1], dt.float32, tag="ar_out", addr_space="Shared")
                  nc.gpsimd.collective_compute(
                      "AllReduce", ALU.add, replica_groups=rg,
                      ins=[part.opt()], outs=[ar_out.opt()])
                  nc.sync.dma_start(ar_sb[:], ar_out[:])
              else:
                  nc.sync.dma_start(ar_sb[:], part[:])
              b_sb = sb.tile([C, 1], dt.float32, tag="b_sb")
              nc.sync.dma_start(b_sb[:], bias[:])
              o_sb = sb.tile([C, 1], dt.float32, tag="o_sb")
              nc.vector.tensor_tensor(out=o_sb[:], in0=ar_sb[:], in1=b_sb[:], op=ALU.add)
              nc.sync.dma_start(score[:], o_sb[:])

    nc.compile()
    return nc


def prep_inputs(inputs, N=N_FULL, H=H_FULL, ncores=NCORES, nlay=NLAY, at_fp8=False):
    """Host-side re-layout of the full inputs into per-core input maps."""
    inp = {k: np.asarray(v) for k, v in inputs.items()}
    NPC = N // ncores
    HT = H // 128
    f32 = np.float32

    feat = inp["feat"].astype(f32)
    src = inp["edge_src"].astype(np.int64)
    dst = inp["edge_dst"].astype(np.int64)

    A_T = np.zeros((N, N), f32)
    np.add.at(A_T, (src, dst), 1.0)
    eps_list = [float(inp["eps0"])] + [float(x) for x in inp["epsR"]]
    diag = np.arange(N)
    at_np_dt = ml_dtypes.float8_e4m3 if at_fp8 else bf16
    at_all = np.empty((nlay, N, N), at_np_dt)
    for i in range(nlay):
        M = A_T.copy()
        M[diag, diag] += 1.0 + eps_list[i]
        at_all[i] = M.astype(at_np_dt)

    featT = np.ascontiguousarray(feat.T).astype(bf16)
    w0a = inp["W0a"].astype(f32).astype(bf16)
    wra = inp["WRa"].astype(f32).astype(bf16)
    wb = np.concatenate([inp["W0b"][None], inp["WRb"]], axis=0).astype(f32).astype(bf16)

    ba = [inp["b0a"]] + [inp["bRa"][i] for i in range(nlay - 1)]
    bb = [inp["b0b"]] + [inp["bRb"][i] for i in range(nlay - 1)]

    def fold(nm, i):
        idx = (lambda x: x) if i == 0 else (lambda x: x[i - 1])
        g, b_, m, v = [idx(inp[nm + s]) for s in ("_g", "_b", "_m", "_v")]
        s = (g / np.sqrt(v + 1e-5)).astype(f32)
        return s, b_, m

    # aff[p, lay*6*HT + stage*HT + m] with stages (s1,t1,s2,t2,s3,t3)
    aff = np.zeros((128, nlay * 6 * HT), f32)
    for i in range(nlay):
        nms = ("bn0a", "bnA0", "bnO0") if i == 0 else ("bnRa", "bnAR", "bnOR")
        s, b_, m = fold(nms[0], i)
        p1s, p1t = s, ((ba[i] - m) * s + b_).astype(f32)
        s, b_, m = fold(nms[1], i)
        p2s, p2t = s, ((bb[i] - m) * s + b_).astype(f32)
        s, b_, m = fold(nms[2], i)
        p3s, p3t = s, (b_ - m * s).astype(f32)
        for mi in range(HT):
            sl = slice(mi * 128, (mi + 1) * 128)
            for j, vec in enumerate((p1s, p1t, p2s, p2t, p3s, p3t)):
                aff[:, i * 6 * HT + j * HT + mi] = vec[sl]

    wp0r = np.ascontiguousarray(
        inp["Wp0"].astype(f32).reshape(N, N, C).transpose(2, 1, 0)).astype(bf16)
    wprr = np.ascontiguousarray(
        inp["WpR"].astype(f32).reshape(nlay, N, H, C).transpose(0, 3, 2, 1)).astype(bf16)
    bias_tot = (inp["bp0"] + inp["bpR"].sum(axis=0)).astype(f32).reshape(C, 1)

    in_maps = []
    for cix in range(ncores):
        sl = slice(cix * NPC, (cix + 1) * NPC)
        in_maps.append({
            "featT": np.ascontiguousarray(featT[:, sl]),
            "w0a": w0a,
            "wra": wra,
            "wb": wb,
            "at": np.ascontiguousarray(at_all[:, :, sl]),
            "wp0": np.ascontiguousarray(wp0r[:, :, sl]),
            "wpr": np.ascontiguousarray(wprr[:, :, :, sl]),
            "aff": aff,
            "bias": bias_tot,
        })
    return in_maps


_CACHE = {}


def _get_program():
    if "nc" not in _CACHE:
        _CACHE["nc"] = build_program()
    return _CACHE["nc"]


def kernel(**inputs):
    nc = _get_program()
    in_maps = prep_inputs(inputs)
    res = run_bass_kernel_spmd(nc, in_maps, list(range(NCORES)))
    return np.ascontiguousarray(res.results[0]["score"].reshape(1, C)).astype(np.float32)

